# revision 15
# baseline (speedup 1.0000x reference)
"""Trainium2 Bass kernel for a 2-layer GRU encoder (TF GRUCell semantics).

Strategy: data-parallel over batch across 8 NeuronCores (4 rows/core, no
collectives — collective latency floors dwarf the per-step budget).

Per core:
- Embedding rows are gathered with indirect DMA, PE-transposed to a
  feature-major layout, and split into bf16 hi/lo pairs.
- Input projections (x_t @ W_x + b) are hoisted out of the scan and
  computed as large GEMMs into DRAM scratch (fp32).
- The sequential scan keeps the recurrent weights resident in SBUF as
  bf16 hi/lo pairs and computes each gate matmul as a 3-term split
  product (W_hi*h_hi + W_hi*h_lo + W_lo*h_hi, fp32 PSUM accumulation),
  which tracks the fp32 reference to ~1e-3 — plain bf16 diverges ~70%
  because this GRU amplifies per-step noise ~1000x over 512 steps, and
  native fp32 matmuls measured 427ns vs 34ns for bf16 pairs.
- The hidden state stays transposed (features on partitions) so the gate
  math uses all 128 DVE/ACT lanes.
- Dynamic (loop-variable) addressing appears only in whole-iteration slab
  DMAs: each engine has a tiny register budget and register-offset APs
  exhaust it fast.

Toolchain notes (pinned walrus build):
- at most ONE sync-wait per instruction -> split extras into NoOps.
- the lower_dve pass crashes on Tile SWDGE programs -> dropped (no
  custom DVE ops used).
- DVE instructions may read at most one PSUM operand.
"""

import numpy as np

B, S, V, E, U = 32, 512, 32000, 256, 1024
NCORES = 8
BC = B // NCORES          # batch rows per core
KH = U // 128             # 8  k-chunks of the hidden dim
KG = 2 * U // 128         # 16 m-chunks of the gate dim
KC = U // 128             # 8  m-chunks of the candidate dim
KE = E // 128             # 2  k-chunks of the embedding dim

_CACHE = {}
TRACE = False
LAST_RESULT = None


# ---------------------------------------------------------------------------
# toolchain workarounds
# ---------------------------------------------------------------------------

def _split_multiwait(nc, max_waits=1):
    """This walrus build accepts at most one sync-wait per instruction;
    split Tile's multi-wait instructions into single-wait NoOps."""
    import concourse.mybir as mybir

    n = 0
    for f in nc.m.functions:
        for bb in f.blocks:
            il = bb.instructions
            i = 0
            while i < len(il):
                inst = il[i]
                si = inst.sync_info
                if (
                    si is not None
                    and si.on_wait is not None
                    and len(si.on_wait) > max_waits
                ):
                    waits = list(si.on_wait)
                    keep = waits[-max_waits:]
                    extra = waits[:-max_waits]
                    nops = []
                    for w in extra:
                        n += 1
                        nop = mybir.InstNoOp(
                            name=f"I-wsplit{n}", engine=inst.engine, ins=[], outs=[]
                        )
                        nop.sync_info = mybir.SyncInfo(on_wait=[w], on_update=[])
                        nops.append(nop)
                    inst.sync_info = mybir.SyncInfo(
                        on_wait=keep, on_update=list(si.on_update or [])
                    )
                    for j, nop in enumerate(nops):
                        il.insert(i + j, nop)
                    i += len(nops)
                i += 1
    return n


_PASSES_PATCHED = False


def _patch_walrus_passes():
    """Drop the lower_dve walrus pass (crashes on Tile SWDGE programs; a
    no-op here since no custom DVE ops are used)."""
    global _PASSES_PATCHED
    if _PASSES_PATCHED:
        return
    _PASSES_PATCHED = True
    from concourse import bass_utils

    orig = bass_utils.bir_verify_and_optimise
    fn = orig.__wrapped__ if hasattr(orig, "__wrapped__") else orig

    def patched(tmpdir, inp="bir.json", outp="file.neff", arch=None, *, dve_root=None):
        orig_run = bass_utils.run_command

        def run_patched(cmd, **kw):
            cmd = [
                c.replace("lower_dve,", "") if isinstance(c, str) else c for c in cmd
            ]
            return orig_run(cmd, **kw)

        bass_utils.run_command = run_patched
        try:
            return fn(tmpdir, inp=inp, outp=outp, arch=arch, dve_root=dve_root)
        finally:
            bass_utils.run_command = orig_run

    bass_utils.bir_verify_and_optimise = patched


# ---------------------------------------------------------------------------
# program builder
# ---------------------------------------------------------------------------

def build_nc(S=S, V=V, unroll=8, nquarters=4, debug=False):
    import concourse.bass as bass
    import concourse.mybir as mybir
    import concourse.tile as tile
    from concourse import bacc
    from concourse.bass import ds
    from concourse.masks import make_identity

    dt = mybir.dt
    f32, bf16 = dt.float32, dt.bfloat16
    P = 128
    QS = S // nquarters
    assert QS % unroll == 0 and S % nquarters == 0
    HCH = min(P, S)        # hoist token-chunk (HCH steps x BC batch cols)
    SIG = mybir.ActivationFunctionType.Sigmoid
    TANH = mybir.ActivationFunctionType.Tanh

    nc = bacc.Bacc("TRN2", target_bir_lowering=False, debug=debug)

    x_d = nc.dram_tensor("x", [BC, S], dt.int32, kind="ExternalInput")
    emb_d = nc.dram_tensor("emb", [V, E], dt.float32, kind="ExternalInput")
    h0_d = [
        nc.dram_tensor("h0_0", [BC, U], dt.float32, kind="ExternalInput"),
        nc.dram_tensor("h0_1", [BC, U], dt.float32, kind="ExternalInput"),
    ]
    Wg_d = [
        nc.dram_tensor("Wg0", [E + U, 2 * U], dt.float32, kind="ExternalInput"),
        nc.dram_tensor("Wg1", [2 * U, 2 * U], dt.float32, kind="ExternalInput"),
    ]
    bg_d = [
        nc.dram_tensor("bg0", [2 * U], dt.float32, kind="ExternalInput"),
        nc.dram_tensor("bg1", [2 * U], dt.float32, kind="ExternalInput"),
    ]
    Wc_d = [
        nc.dram_tensor("Wc0", [E + U, U], dt.float32, kind="ExternalInput"),
        nc.dram_tensor("Wc1", [2 * U, U], dt.float32, kind="ExternalInput"),
    ]
    bc_d = [
        nc.dram_tensor("bc0", [U], dt.float32, kind="ExternalInput"),
        nc.dram_tensor("bc1", [U], dt.float32, kind="ExternalInput"),
    ]
    y1_d = nc.dram_tensor("y1", [BC, S, U], dt.float32, kind="ExternalOutput")
    s_d = [
        nc.dram_tensor("s0", [BC, U], dt.float32, kind="ExternalOutput"),
        nc.dram_tensor("s1", [BC, U], dt.float32, kind="ExternalOutput"),
    ]
    # DRAM scratch: hoisted projections + layer-0 outputs, feature-major
    gx_d = nc.dram_tensor("gx_scr", [P, KG, S, BC], dt.float32)
    cx_d = nc.dram_tensor("cx_scr", [P, KC, S, BC], dt.float32)
    y0_d = nc.dram_tensor("y0_scr", [P, KC, S, BC], dt.float32)

    from contextlib import ExitStack

    with tile.TileContext(nc) as tc, ExitStack() as ctx:
        const = ctx.enter_context(tc.tile_pool(name="const", bufs=1))
        wpool = ctx.enter_context(tc.tile_pool(name="w", bufs=1))
        sp = ctx.enter_context(tc.tile_pool(name="sp", bufs=2))
        qpool = ctx.enter_context(tc.tile_pool(name="q", bufs=1))
        sp3 = ctx.enter_context(tc.tile_pool(name="sp3", bufs=3))
        pp = ctx.enter_context(tc.tile_pool(name="pp", bufs=2, space="PSUM"))

        idf = const.tile([P, P], f32, tag="idf")
        make_identity(nc, idf[:])

        # persistent state
        xep = const.tile([P, KE, 2, S, BC], bf16, tag="xep")   # hi/lo pairs
        hT = const.tile([P, KH, 1, BC], f32, tag="hT")
        hpair = const.tile([P, KH, 2, BC], bf16, tag="hpair")

        # ---- biases, transposed to [128, nm] ----
        def load_bias_T(dram, nm, tag):
            stage = sp.tile([nm, P], f32, tag="bstage")
            nc.sync.dma_start(stage[:], dram[:])
            ps = pp.tile([P, nm], f32, tag="ptr")
            nc.tensor.transpose(ps[:], stage[:], idf[0:nm, 0:nm])
            out = const.tile([P, nm], f32, tag=tag)
            nc.vector.tensor_copy(out[:], ps[:])
            return out

        bgT = [load_bias_T(bg_d[l], KG, f"bgT{l}") for l in (0, 1)]
        bcT = [load_bias_T(bc_d[l], KC, f"bcT{l}") for l in (0, 1)]

        # ---- embedding gather + transpose + split into xep ----
        TT = min(P, S)
        for b in range(BC):
            for g in range(S // TT):
                t0 = g * TT
                idx = sp3.tile([TT, 1], dt.int32, tag="idx")
                nc.sync.dma_start(idx[:], x_d[b, t0:t0 + TT])
                xe = sp3.tile([TT, E], f32, tag="xe")
                nc.gpsimd.indirect_dma_start(
                    out=xe[:],
                    out_offset=None,
                    in_=emb_d[:],
                    in_offset=bass.IndirectOffsetOnAxis(ap=idx[:, :1], axis=0),
                )
                for e in range(KE):
                    ps = pp.tile([P, TT], f32, tag="ptr")
                    nc.tensor.transpose(
                        ps[:], xe[:, e * P:(e + 1) * P], idf[0:TT, 0:TT]
                    )
                    hi = xep[:, e, 0, t0:t0 + TT, b]
                    nc.vector.tensor_copy(hi, ps[:])
                    nc.vector.tensor_sub(xep[:, e, 1, t0:t0 + TT, b], ps[:], hi)

        # ---- split-weight loader: DRAM f32 rows -> bf16 hi/lo tiles ----
        def load_wpair(dram, row0, nk, nm, tag):
            whi = wpool.tile([P, nk, nm, P], bf16, tag=f"{tag}_hi")
            wlo = wpool.tile([P, nk, nm, P], bf16, tag=f"{tag}_lo")
            for k in range(nk):
                stage = qpool.tile([P, nm, P], f32, tag="wstage")
                r = row0 + k * P
                nc.sync.dma_start(stage[:], dram[r:r + P, :])
                nc.vector.tensor_copy(whi[:, k], stage[:])
                nc.vector.tensor_sub(wlo[:, k], stage[:], whi[:, k])
            return whi, wlo

        def h_init(l):
            stage = qpool.tile([BC, U], f32, tag="hstage")
            nc.sync.dma_start(stage[:], h0_d[l][:])
            for c in range(KH):
                ps = pp.tile([P, BC], f32, tag="ptr")
                nc.tensor.transpose(
                    ps[:], stage[:, c * P:(c + 1) * P], idf[0:BC, 0:BC]
                )
                nc.vector.tensor_copy(hT[:, c, 0, :], ps[:])
            nc.vector.tensor_copy(hpair[:, :, 0:1, :], hT[:])
            nc.vector.tensor_sub(hpair[:, :, 1:2, :], hT[:], hpair[:, :, 0:1, :])

        def dump_state(l):
            stage = qpool.tile([BC, KH, P], f32, tag="sstage")
            for c in range(KH):
                ps = pp.tile([BC, P], f32, tag="ptr")
                nc.tensor.transpose(ps[:], hT[:, c, 0, :], idf[:])
                nc.vector.tensor_copy(stage[:, c, :], ps[:])
            nc.sync.dma_start(s_d[l][:], stage[:])

        def mm3(psum, whi, wlo, rp, k, m, first, last):
            """psum[:, :] += Whi.T @ (hi|lo) + Wlo.T @ hi for one k-chunk.
            rp: [P, nk, 2, N...] bf16 pair source; psum cols packed [2, N]."""
            nc.tensor.matmul(
                psum[0], whi[:, k, m, :], rp[0],
                start=first, stop=False, skip_group_check=True,
            )
            nc.tensor.matmul(
                psum[1], wlo[:, k, m, :], rp[1],
                start=False, stop=last, skip_group_check=True,
            )

        for l in (0, 1):
            nin = E if l == 0 else U
            nkin = nin // P

            # ============ hoist x_t @ W_x + b for the whole sequence ====
            wxhi, wxlo = load_wpair(Wg_d[l], 0, nkin, KG, "wa")
            cxhi, cxlo = load_wpair(Wc_d[l], 0, nkin, KC, "wb")
            h_init(l)

            for hc in range(S // HCH):
                t0 = hc * HCH
                if l == 1:
                    y0q = qpool.tile([P, KH, HCH, BC], f32, tag="y0q")
                    nc.sync.dma_start(
                        y0q[:], y0_d[:, :, t0:t0 + HCH, :]
                    )
                    y0p = qpool.tile([P, KH, 2, HCH, BC], bf16, tag="y0p")
                    nc.vector.tensor_copy(y0p[:, :, 0, :, :], y0q[:])
                    nc.vector.tensor_sub(
                        y0p[:, :, 1, :, :], y0q[:], y0p[:, :, 0, :, :]
                    )
                for (wh, wl, nm, bias, dst) in (
                    (wxhi, wxlo, KG, bgT[l], gx_d),
                    (cxhi, cxlo, KC, bcT[l], cx_d),
                ):
                    for m in range(nm):
                        ph = pp.tile([P, HCH, BC], f32, tag="ph")
                        for k in range(nkin):
                            if l == 0:
                                rhi = xep[:, k, 0, t0:t0 + HCH, :]
                                rlo = xep[:, k, 1, t0:t0 + HCH, :]
                            else:
                                rhi = y0p[:, k, 0, :, :]
                                rlo = y0p[:, k, 1, :, :]
                            first = k == 0
                            last = k == nkin - 1
                            nc.tensor.matmul(
                                ph[:], wh[:, k, m, :], rhi,
                                start=first, stop=False, skip_group_check=True,
                            )
                            nc.tensor.matmul(
                                ph[:], wh[:, k, m, :], rlo,
                                start=False, stop=False, skip_group_check=True,
                            )
                            nc.tensor.matmul(
                                ph[:], wl[:, k, m, :], rhi,
                                start=False, stop=last, skip_group_check=True,
                            )
                        stage = sp.tile([P, HCH, BC], f32, tag="hstg")
                        nc.vector.scalar_tensor_tensor(
                            out=stage[:],
                            in0=ph[:],
                            scalar=1.0,
                            in1=bias[:, m:m + 1].to_broadcast([P, HCH, BC]),
                            op0=mybir.AluOpType.mult,
                            op1=mybir.AluOpType.add,
                        )
                        nc.sync.dma_start(dst[:, m, t0:t0 + HCH, :], stage[:])

            # ============ recurrent scan ================================
            ghi, glo = load_wpair(Wg_d[l], nin, KH, KG, "wa")
            chi, clo = load_wpair(Wc_d[l], nin, KH, KC, "wb")

            for q in range(nquarters):
                t0 = q * QS
                if l == 1:
                    y1q = qpool.tile([P, KC, QS, BC], f32, tag="y1q")
                with tc.For_i(
                    0, QS, unroll, hint_engines=(mybir.EngineType.PE,)
                ) as iv:
                    gx8 = sp.tile([P, KG, unroll, BC], f32, tag="gx8")
                    nc.sync.dma_start(
                        gx8[:], gx_d[:, :, ds(t0 + iv, unroll), :]
                    )
                    cx8 = sp.tile([P, KC, unroll, BC], f32, tag="cx8")
                    nc.sync.dma_start(
                        cx8[:], cx_d[:, :, ds(t0 + iv, unroll), :]
                    )
                    y8 = sp.tile([P, KC, unroll, BC], f32, tag="y8")
                    for ui in range(unroll):
                        psg = pp.tile([P, KG, 2, BC], f32, tag="pgi")
                        for m in range(KC):          # r-half
                            for k in range(KH):
                                mm3(
                                    (psg[:, m, :, :], psg[:, m, 0:1, :]),
                                    ghi, glo,
                                    (hpair[:, k, :, :], hpair[:, k, 0:1, :]),
                                    k, m, k == 0, k == KH - 1,
                                )
                        t1 = sp.tile([P, KC, 1, BC], f32, tag="t1")
                        nc.vector.tensor_add(
                            t1[:], psg[:, 0:KC, 1:2, :],
                            gx8[:, 0:KC, ui:ui + 1, :],
                        )
                        gir = sp.tile([P, KC, 1, BC], f32, tag="gir")
                        nc.vector.tensor_add(gir[:], psg[:, 0:KC, 0:1, :], t1[:])
                        rg = sp.tile([P, KC, 1, BC], f32, tag="rg")
                        nc.scalar.activation(rg[:], gir[:], SIG)
                        rh = sp.tile([P, KH, 1, BC], f32, tag="rh")
                        nc.vector.tensor_mul(rh[:], rg[:], hT[:])
                        rp = sp.tile([P, KH, 2, BC], bf16, tag="rp")
                        nc.vector.tensor_copy(rp[:, :, 0:1, :], rh[:])
                        nc.vector.tensor_sub(rp[:, :, 1:2, :], rh[:], rp[:, :, 0:1, :])
                        for m in range(KC, KG):      # u-half
                            for k in range(KH):
                                mm3(
                                    (psg[:, m, :, :], psg[:, m, 0:1, :]),
                                    ghi, glo,
                                    (hpair[:, k, :, :], hpair[:, k, 0:1, :]),
                                    k, m, k == 0, k == KH - 1,
                                )
                        psc = pp.tile([P, KC, 2, BC], f32, tag="pc")
                        for m in range(KC):          # candidate
                            for k in range(KH):
                                mm3(
                                    (psc[:, m, :, :], psc[:, m, 0:1, :]),
                                    chi, clo,
                                    (rp[:, k, :, :], rp[:, k, 0:1, :]),
                                    k, m, k == 0, k == KH - 1,
                                )
                        t2 = sp.tile([P, KC, 1, BC], f32, tag="t2")
                        nc.vector.tensor_add(
                            t2[:], psg[:, KC:KG, 1:2, :],
                            gx8[:, KC:KG, ui:ui + 1, :],
                        )
                        giu = sp.tile([P, KC, 1, BC], f32, tag="giu")
                        nc.vector.tensor_add(giu[:], psg[:, KC:KG, 0:1, :], t2[:])
                        ug = sp.tile([P, KC, 1, BC], f32, tag="ug")
                        nc.scalar.activation(ug[:], giu[:], SIG)
                        t3 = sp.tile([P, KC, 1, BC], f32, tag="t3")
                        nc.vector.tensor_add(
                            t3[:], psc[:, :, 1:2, :], cx8[:, :, ui:ui + 1, :]
                        )
                        cpre = sp.tile([P, KC, 1, BC], f32, tag="cpre")
                        nc.vector.tensor_add(cpre[:], psc[:, :, 0:1, :], t3[:])
                        cg = sp.tile([P, KC, 1, BC], f32, tag="cg")
                        nc.scalar.activation(cg[:], cpre[:], TANH)
                        # h' = c + u*(h - c)
                        t4 = sp.tile([P, KC, 1, BC], f32, tag="t4")
                        nc.vector.tensor_sub(t4[:], hT[:], cg[:])
                        t5 = sp.tile([P, KC, 1, BC], f32, tag="t5")
                        nc.vector.tensor_mul(t5[:], ug[:], t4[:])
                        nc.vector.tensor_add(hT[:], cg[:], t5[:])
                        nc.vector.tensor_copy(hpair[:, :, 0:1, :], hT[:])
                        nc.vector.tensor_sub(
                            hpair[:, :, 1:2, :], hT[:], hpair[:, :, 0:1, :]
                        )
                        nc.vector.tensor_copy(y8[:, :, ui:ui + 1, :], hT[:])
                    if l == 0:
                        nc.sync.dma_start(
                            y0_d[:, :, ds(t0 + iv, unroll), :], y8[:]
                        )
                    else:
                        nc.sync.dma_start(
                            y1q[:, :, ds(iv, unroll), :], y8[:]
                        )

                # ---- dump y1 quarter: transpose back to [b, t, u] ----
                if l == 1:
                    TQ = min(P, QS)
                    for c in range(KC):
                        for b in range(BC):
                            for tt in range(QS // TQ):
                                ps = pp.tile([TQ, P], f32, tag="ptr")
                                nc.tensor.transpose(
                                    ps[:],
                                    y1q[:, c, tt * TQ:(tt + 1) * TQ, b],
                                    idf[:],
                                )
                                stage = sp.tile([TQ, P], f32, tag="ystage")
                                nc.vector.tensor_copy(stage[:], ps[:])
                                nc.sync.dma_start(
                                    y1_d[
                                        b,
                                        t0 + tt * TQ:t0 + (tt + 1) * TQ,
                                        c * P:(c + 1) * P,
                                    ],
                                    stage[:],
                                )

            dump_state(l)

    return nc


def _get_program():
    key = "full"
    if key not in _CACHE:
        _patch_walrus_passes()
        nc = build_nc()
        nc.finalize()
        _CACHE[key] = nc
    return _CACHE[key]


def kernel(**inputs):
    global LAST_RESULT
    _patch_walrus_passes()
    from concourse.bass_utils import run_bass_kernel_spmd

    ins = {k: np.asarray(v) for k, v in inputs.items()}
    x = ins["x"].astype(np.int32)

    nc = _get_program()
    rep = {}
    for name in ("emb", "Wg0", "bg0", "Wc0", "bc0", "Wg1", "bg1", "Wc1", "bc1"):
        rep[name] = np.ascontiguousarray(ins[name].astype(np.float32))
    in_maps = []
    for c in range(NCORES):
        sl = slice(c * BC, (c + 1) * BC)
        m = dict(rep)
        m["x"] = np.ascontiguousarray(x[sl])
        m["h0_0"] = np.ascontiguousarray(ins["h0_0"][sl].astype(np.float32))
        m["h0_1"] = np.ascontiguousarray(ins["h0_1"][sl].astype(np.float32))
        in_maps.append(m)

    res = run_bass_kernel_spmd(nc, in_maps, list(range(NCORES)), trace=TRACE)
    LAST_RESULT = res
    y1 = np.concatenate([res.results[c]["y1"] for c in range(NCORES)], axis=0)
    s0 = np.concatenate([res.results[c]["s0"] for c in range(NCORES)], axis=0)
    s1 = np.concatenate([res.results[c]["s1"] for c in range(NCORES)], axis=0)
    return y1, s0, s1


# revision 17
# speedup vs baseline: 1.0300x; 1.0300x over previous
"""Trainium2 Bass kernel for a 2-layer GRU encoder (TF GRUCell semantics).

Strategy: data-parallel over batch across 8 NeuronCores (4 rows/core, no
collectives — collective latency floors dwarf the per-step budget).

Per core:
- Embedding rows are gathered with indirect DMA, PE-transposed to a
  feature-major layout, and split into bf16 hi/lo pairs.
- Input projections (x_t @ W_x + b) are hoisted out of the scan and
  computed as large GEMMs into DRAM scratch (fp32).
- The sequential scan keeps the recurrent weights resident in SBUF as
  bf16 hi/lo pairs and computes each gate matmul as a 3-term split
  product (W_hi*h_hi + W_hi*h_lo + W_lo*h_hi, fp32 PSUM accumulation),
  which tracks the fp32 reference to ~1e-3 — plain bf16 diverges ~70%
  because this GRU amplifies per-step noise ~1000x over 512 steps, and
  native fp32 matmuls measured 427ns vs 34ns for bf16 pairs.
- The hidden state stays transposed (features on partitions) so the gate
  math uses all 128 DVE/ACT lanes.
- Dynamic (loop-variable) addressing appears only in whole-iteration slab
  DMAs: each engine has a tiny register budget and register-offset APs
  exhaust it fast.

Toolchain notes (pinned walrus build):
- at most ONE sync-wait per instruction -> split extras into NoOps.
- the lower_dve pass crashes on Tile SWDGE programs -> dropped (no
  custom DVE ops used).
- DVE instructions may read at most one PSUM operand.
"""

import numpy as np

B, S, V, E, U = 32, 512, 32000, 256, 1024
NCORES = 8
BC = B // NCORES          # batch rows per core
KH = U // 128             # 8  k-chunks of the hidden dim
KG = 2 * U // 128         # 16 m-chunks of the gate dim
KC = U // 128             # 8  m-chunks of the candidate dim
KE = E // 128             # 2  k-chunks of the embedding dim

_CACHE = {}
TRACE = False
LAST_RESULT = None


# ---------------------------------------------------------------------------
# toolchain workarounds
# ---------------------------------------------------------------------------

def _split_multiwait(nc, max_waits=1):
    """This walrus build accepts at most one sync-wait per instruction;
    split Tile's multi-wait instructions into single-wait NoOps."""
    import concourse.mybir as mybir

    n = 0
    for f in nc.m.functions:
        for bb in f.blocks:
            il = bb.instructions
            i = 0
            while i < len(il):
                inst = il[i]
                si = inst.sync_info
                if (
                    si is not None
                    and si.on_wait is not None
                    and len(si.on_wait) > max_waits
                ):
                    waits = list(si.on_wait)
                    keep = waits[-max_waits:]
                    extra = waits[:-max_waits]
                    nops = []
                    for w in extra:
                        n += 1
                        nop = mybir.InstNoOp(
                            name=f"I-wsplit{n}", engine=inst.engine, ins=[], outs=[]
                        )
                        nop.sync_info = mybir.SyncInfo(on_wait=[w], on_update=[])
                        nops.append(nop)
                    inst.sync_info = mybir.SyncInfo(
                        on_wait=keep, on_update=list(si.on_update or [])
                    )
                    for j, nop in enumerate(nops):
                        il.insert(i + j, nop)
                    i += len(nops)
                i += 1
    return n


_PASSES_PATCHED = False


def _patch_walrus_passes():
    """Drop the lower_dve walrus pass (crashes on Tile SWDGE programs; a
    no-op here since no custom DVE ops are used)."""
    global _PASSES_PATCHED
    if _PASSES_PATCHED:
        return
    _PASSES_PATCHED = True
    from concourse import bass_utils

    orig = bass_utils.bir_verify_and_optimise
    fn = orig.__wrapped__ if hasattr(orig, "__wrapped__") else orig

    def patched(tmpdir, inp="bir.json", outp="file.neff", arch=None, *, dve_root=None):
        orig_run = bass_utils.run_command

        def run_patched(cmd, **kw):
            cmd = [
                c.replace("lower_dve,", "") if isinstance(c, str) else c for c in cmd
            ]
            return orig_run(cmd, **kw)

        bass_utils.run_command = run_patched
        try:
            return fn(tmpdir, inp=inp, outp=outp, arch=arch, dve_root=dve_root)
        finally:
            bass_utils.run_command = orig_run

    bass_utils.bir_verify_and_optimise = patched


# ---------------------------------------------------------------------------
# program builder
# ---------------------------------------------------------------------------

def build_nc(S=S, V=V, unroll=16, nquarters=4, debug=False):
    import concourse.bass as bass
    import concourse.mybir as mybir
    import concourse.tile as tile
    from concourse import bacc
    from concourse.bass import ds
    from concourse.masks import make_identity

    dt = mybir.dt
    f32, bf16 = dt.float32, dt.bfloat16
    P = 128
    QS = S // nquarters
    assert QS % unroll == 0 and S % nquarters == 0
    HCH = min(P, S)        # hoist token-chunk (HCH steps x BC batch cols)
    SIG = mybir.ActivationFunctionType.Sigmoid
    TANH = mybir.ActivationFunctionType.Tanh

    nc = bacc.Bacc("TRN2", target_bir_lowering=False, debug=debug)

    x_d = nc.dram_tensor("x", [BC, S], dt.int32, kind="ExternalInput")
    emb_d = nc.dram_tensor("emb", [V, E], dt.float32, kind="ExternalInput")
    h0_d = [
        nc.dram_tensor("h0_0", [BC, U], dt.float32, kind="ExternalInput"),
        nc.dram_tensor("h0_1", [BC, U], dt.float32, kind="ExternalInput"),
    ]
    Wg_d = [
        nc.dram_tensor("Wg0", [E + U, 2 * U], dt.float32, kind="ExternalInput"),
        nc.dram_tensor("Wg1", [2 * U, 2 * U], dt.float32, kind="ExternalInput"),
    ]
    bg_d = [
        nc.dram_tensor("bg0", [2 * U], dt.float32, kind="ExternalInput"),
        nc.dram_tensor("bg1", [2 * U], dt.float32, kind="ExternalInput"),
    ]
    Wc_d = [
        nc.dram_tensor("Wc0", [E + U, U], dt.float32, kind="ExternalInput"),
        nc.dram_tensor("Wc1", [2 * U, U], dt.float32, kind="ExternalInput"),
    ]
    bc_d = [
        nc.dram_tensor("bc0", [U], dt.float32, kind="ExternalInput"),
        nc.dram_tensor("bc1", [U], dt.float32, kind="ExternalInput"),
    ]
    y1_d = nc.dram_tensor("y1", [BC, S, U], dt.float32, kind="ExternalOutput")
    s_d = [
        nc.dram_tensor("s0", [BC, U], dt.float32, kind="ExternalOutput"),
        nc.dram_tensor("s1", [BC, U], dt.float32, kind="ExternalOutput"),
    ]
    # DRAM scratch: hoisted projections + layer-0 outputs, feature-major
    gx_d = nc.dram_tensor("gx_scr", [P, KG, S, BC], dt.float32)
    cx_d = nc.dram_tensor("cx_scr", [P, KC, S, BC], dt.float32)
    y0_d = nc.dram_tensor("y0_scr", [P, KC, S, BC], dt.float32)

    from contextlib import ExitStack

    with tile.TileContext(nc) as tc, ExitStack() as ctx:
        const = ctx.enter_context(tc.tile_pool(name="const", bufs=1))
        wpool = ctx.enter_context(tc.tile_pool(name="w", bufs=1))
        sp = ctx.enter_context(tc.tile_pool(name="sp", bufs=2))
        qpool = ctx.enter_context(tc.tile_pool(name="q", bufs=1))
        sp3 = ctx.enter_context(tc.tile_pool(name="sp3", bufs=3))
        pp = ctx.enter_context(tc.tile_pool(name="pp", bufs=2, space="PSUM"))
        # pp carries only the small "ptr" transpose tiles; matmul psum tiles
        # live in per-phase pools so the 8 PSUM banks are never oversubscribed

        idf = const.tile([P, P], f32, tag="idf")
        make_identity(nc, idf[:])

        # persistent state
        xep = const.tile([P, KE, 2, S, BC], bf16, tag="xep")   # hi/lo pairs
        hT = const.tile([P, KH, 1, BC], f32, tag="hT")
        hpair = const.tile([P, KH, 2, BC], bf16, tag="hpair")

        # ---- biases, transposed to [128, nm] ----
        def load_bias_T(dram, nm, tag):
            stage = sp.tile([nm, P], f32, tag="bstage")
            nc.sync.dma_start(stage[:], dram[:])
            ps = pp.tile([P, nm], f32, tag="ptr")
            nc.tensor.transpose(ps[:], stage[:], idf[0:nm, 0:nm])
            out = const.tile([P, nm], f32, tag=tag)
            nc.vector.tensor_copy(out[:], ps[:])
            return out

        bgT = [load_bias_T(bg_d[l], KG, f"bgT{l}") for l in (0, 1)]
        bcT = [load_bias_T(bc_d[l], KC, f"bcT{l}") for l in (0, 1)]

        # ---- embedding gather + transpose + split into xep ----
        TT = min(P, S)
        for b in range(BC):
            for g in range(S // TT):
                t0 = g * TT
                idx = sp3.tile([TT, 1], dt.int32, tag="idx")
                nc.sync.dma_start(idx[:], x_d[b, t0:t0 + TT])
                xe = sp3.tile([TT, E], f32, tag="xe")
                nc.gpsimd.indirect_dma_start(
                    out=xe[:],
                    out_offset=None,
                    in_=emb_d[:],
                    in_offset=bass.IndirectOffsetOnAxis(ap=idx[:, :1], axis=0),
                )
                for e in range(KE):
                    ps = pp.tile([P, TT], f32, tag="ptr")
                    nc.tensor.transpose(
                        ps[:], xe[:, e * P:(e + 1) * P], idf[0:TT, 0:TT]
                    )
                    hi = xep[:, e, 0, t0:t0 + TT, b]
                    nc.vector.tensor_copy(hi, ps[:])
                    nc.vector.tensor_sub(xep[:, e, 1, t0:t0 + TT, b], ps[:], hi)

        # ---- split-weight loader: DRAM f32 rows -> bf16 hi/lo tiles ----
        def load_wpair(dram, row0, nk, nm, tag):
            whi = wpool.tile([P, nk, nm, P], bf16, tag=f"{tag}_hi")
            wlo = wpool.tile([P, nk, nm, P], bf16, tag=f"{tag}_lo")
            for k in range(nk):
                stage = qpool.tile([P, nm, P], f32, tag="wstage")
                r = row0 + k * P
                nc.sync.dma_start(stage[:], dram[r:r + P, :])
                nc.vector.tensor_copy(whi[:, k], stage[:])
                nc.vector.tensor_sub(wlo[:, k], stage[:], whi[:, k])
            return whi, wlo

        def h_init(l):
            stage = qpool.tile([BC, U], f32, tag="hstage")
            nc.sync.dma_start(stage[:], h0_d[l][:])
            for c in range(KH):
                ps = pp.tile([P, BC], f32, tag="ptr")
                nc.tensor.transpose(
                    ps[:], stage[:, c * P:(c + 1) * P], idf[0:BC, 0:BC]
                )
                nc.vector.tensor_copy(hT[:, c, 0, :], ps[:])
            nc.vector.tensor_copy(hpair[:, :, 0:1, :], hT[:])
            nc.vector.tensor_sub(hpair[:, :, 1:2, :], hT[:], hpair[:, :, 0:1, :])

        def dump_state(l):
            stage = qpool.tile([BC, KH, P], f32, tag="sstage")
            for c in range(KH):
                ps = pp.tile([BC, P], f32, tag="ptr")
                nc.tensor.transpose(ps[:], hT[:, c, 0, :], idf[:])
                nc.vector.tensor_copy(stage[:, c, :], ps[:])
            nc.sync.dma_start(s_d[l][:], stage[:])

        def mm3(psum, whi, wlo, rp, k, m, first, last):
            """psum[:, :] += Whi.T @ (hi|lo) + Wlo.T @ hi for one k-chunk.
            rp: [P, nk, 2, N...] bf16 pair source; psum cols packed [2, N]."""
            nc.tensor.matmul(
                psum[0], whi[:, k, m, :], rp[0],
                start=first, stop=False, skip_group_check=True,
            )
            nc.tensor.matmul(
                psum[1], wlo[:, k, m, :], rp[1],
                start=False, stop=last, skip_group_check=True,
            )

        for l in (0, 1):
            nin = E if l == 0 else U
            nkin = nin // P

            # ============ hoist x_t @ W_x + b for the whole sequence ====
            wxhi, wxlo = load_wpair(Wg_d[l], 0, nkin, KG, "wa")
            cxhi, cxlo = load_wpair(Wc_d[l], 0, nkin, KC, "wb")
            h_init(l)

            hoist_ctx = ExitStack()
            pph = hoist_ctx.enter_context(
                tc.tile_pool(name="pph", bufs=2, space="PSUM")
            )
            for hc in range(S // HCH):
                t0 = hc * HCH
                if l == 1:
                    y0q = qpool.tile([P, KH, HCH, BC], f32, tag="y0q")
                    nc.sync.dma_start(
                        y0q[:], y0_d[:, :, t0:t0 + HCH, :]
                    )
                    y0p = qpool.tile([P, KH, 2, HCH, BC], bf16, tag="y0p")
                    nc.vector.tensor_copy(y0p[:, :, 0, :, :], y0q[:])
                    nc.vector.tensor_sub(
                        y0p[:, :, 1, :, :], y0q[:], y0p[:, :, 0, :, :]
                    )
                for (wh, wl, nm, bias, dst) in (
                    (wxhi, wxlo, KG, bgT[l], gx_d),
                    (cxhi, cxlo, KC, bcT[l], cx_d),
                ):
                    for m in range(nm):
                        ph = pph.tile([P, HCH, BC], f32, tag="ph")
                        for k in range(nkin):
                            if l == 0:
                                rhi = xep[:, k, 0, t0:t0 + HCH, :]
                                rlo = xep[:, k, 1, t0:t0 + HCH, :]
                            else:
                                rhi = y0p[:, k, 0, :, :]
                                rlo = y0p[:, k, 1, :, :]
                            first = k == 0
                            last = k == nkin - 1
                            nc.tensor.matmul(
                                ph[:], wh[:, k, m, :], rhi,
                                start=first, stop=False, skip_group_check=True,
                            )
                            nc.tensor.matmul(
                                ph[:], wh[:, k, m, :], rlo,
                                start=False, stop=False, skip_group_check=True,
                            )
                            nc.tensor.matmul(
                                ph[:], wl[:, k, m, :], rhi,
                                start=False, stop=last, skip_group_check=True,
                            )
                        stage = sp.tile([P, HCH, BC], f32, tag="hstg")
                        nc.vector.scalar_tensor_tensor(
                            out=stage[:],
                            in0=ph[:],
                            scalar=1.0,
                            in1=bias[:, m:m + 1].to_broadcast([P, HCH, BC]),
                            op0=mybir.AluOpType.mult,
                            op1=mybir.AluOpType.add,
                        )
                        nc.sync.dma_start(dst[:, m, t0:t0 + HCH, :], stage[:])

            hoist_ctx.close()

            # ============ recurrent scan ================================
            ghi, glo = load_wpair(Wg_d[l], nin, KH, KG, "wa")
            chi, clo = load_wpair(Wc_d[l], nin, KH, KC, "wb")

            scan_ctx = ExitStack()
            pps = scan_ctx.enter_context(
                tc.tile_pool(name="pps", bufs=2, space="PSUM")
            )
            for q in range(nquarters):
                t0 = q * QS
                if l == 1:
                    y1q = qpool.tile([P, KC, QS, BC], f32, tag="y1q")
                with tc.For_i(
                    0, QS, unroll, hint_engines=(mybir.EngineType.PE,)
                ) as iv:
                    gx8 = sp.tile([P, KG, unroll, BC], f32, tag="gx8")
                    nc.sync.dma_start(
                        gx8[:], gx_d[:, :, ds(t0 + iv, unroll), :]
                    )
                    cx8 = sp.tile([P, KC, unroll, BC], f32, tag="cx8")
                    nc.sync.dma_start(
                        cx8[:], cx_d[:, :, ds(t0 + iv, unroll), :]
                    )
                    y8 = sp.tile([P, KC, unroll, BC], f32, tag="y8")
                    for ui in range(unroll):
                        # r and u halves in separate PSUM banks so the u-half
                        # matmuls don't serialize against the DVE reads of the
                        # r-half (same-bank PE-write/DVE-read is serialized)
                        pgr = pps.tile([P, KC, 2, BC], f32, tag="pgr")
                        for m in range(KC):          # r-half
                            for k in range(KH):
                                mm3(
                                    (pgr[:, m, :, :], pgr[:, m, 0:1, :]),
                                    ghi, glo,
                                    (hpair[:, k, :, :], hpair[:, k, 0:1, :]),
                                    k, m, k == 0, k == KH - 1,
                                )
                        pgu = pps.tile([P, KC, 2, BC], f32, tag="pgu")
                        for m in range(KC, KG):      # u-half
                            for k in range(KH):
                                mm3(
                                    (pgu[:, m - KC, :, :], pgu[:, m - KC, 0:1, :]),
                                    ghi, glo,
                                    (hpair[:, k, :, :], hpair[:, k, 0:1, :]),
                                    k, m, k == 0, k == KH - 1,
                                )
                        t1 = sp.tile([P, KC, 1, BC], f32, tag="t1")
                        nc.vector.tensor_add(
                            t1[:], pgr[:, :, 1:2, :],
                            gx8[:, 0:KC, ui:ui + 1, :],
                        )
                        gir = sp.tile([P, KC, 1, BC], f32, tag="gir")
                        nc.vector.tensor_add(gir[:], pgr[:, :, 0:1, :], t1[:])
                        rg = sp.tile([P, KC, 1, BC], f32, tag="rg")
                        nc.scalar.activation(rg[:], gir[:], SIG)
                        rh = sp.tile([P, KH, 1, BC], f32, tag="rh")
                        nc.vector.tensor_mul(rh[:], rg[:], hT[:])
                        rp = sp.tile([P, KH, 2, BC], bf16, tag="rp")
                        nc.vector.tensor_copy(rp[:, :, 0:1, :], rh[:])
                        nc.vector.tensor_sub(rp[:, :, 1:2, :], rh[:], rp[:, :, 0:1, :])
                        t2 = sp.tile([P, KC, 1, BC], f32, tag="t2")
                        nc.vector.tensor_add(
                            t2[:], pgu[:, :, 1:2, :],
                            gx8[:, KC:KG, ui:ui + 1, :],
                        )
                        giu = sp.tile([P, KC, 1, BC], f32, tag="giu")
                        nc.vector.tensor_add(giu[:], pgu[:, :, 0:1, :], t2[:])
                        ug = sp.tile([P, KC, 1, BC], f32, tag="ug")
                        nc.scalar.activation(ug[:], giu[:], SIG)
                        psc = pps.tile([P, KC, 2, BC], f32, tag="pc")
                        for m in range(KC):          # candidate
                            for k in range(KH):
                                mm3(
                                    (psc[:, m, :, :], psc[:, m, 0:1, :]),
                                    chi, clo,
                                    (rp[:, k, :, :], rp[:, k, 0:1, :]),
                                    k, m, k == 0, k == KH - 1,
                                )
                        t3 = sp.tile([P, KC, 1, BC], f32, tag="t3")
                        nc.vector.tensor_add(
                            t3[:], psc[:, :, 1:2, :], cx8[:, :, ui:ui + 1, :]
                        )
                        cpre = sp.tile([P, KC, 1, BC], f32, tag="cpre")
                        nc.vector.tensor_add(cpre[:], psc[:, :, 0:1, :], t3[:])
                        cg = sp.tile([P, KC, 1, BC], f32, tag="cg")
                        nc.scalar.activation(cg[:], cpre[:], TANH)
                        # h' = c + u*(h - c)
                        t4 = sp.tile([P, KC, 1, BC], f32, tag="t4")
                        nc.vector.tensor_sub(t4[:], hT[:], cg[:])
                        t5 = sp.tile([P, KC, 1, BC], f32, tag="t5")
                        nc.vector.tensor_mul(t5[:], ug[:], t4[:])
                        nc.vector.tensor_add(hT[:], cg[:], t5[:])
                        nc.vector.tensor_copy(hpair[:, :, 0:1, :], hT[:])
                        nc.vector.tensor_sub(
                            hpair[:, :, 1:2, :], hT[:], hpair[:, :, 0:1, :]
                        )
                        nc.vector.tensor_copy(y8[:, :, ui:ui + 1, :], hT[:])
                    if l == 0:
                        nc.sync.dma_start(
                            y0_d[:, :, ds(t0 + iv, unroll), :], y8[:]
                        )
                    else:
                        nc.sync.dma_start(
                            y1q[:, :, ds(iv, unroll), :], y8[:]
                        )

                # ---- dump y1 quarter: transpose back to [b, t, u] ----
                if l == 1:
                    TQ = min(P, QS)
                    for c in range(KC):
                        for b in range(BC):
                            for tt in range(QS // TQ):
                                ps = pp.tile([TQ, P], f32, tag="ptr")
                                nc.tensor.transpose(
                                    ps[:],
                                    y1q[:, c, tt * TQ:(tt + 1) * TQ, b],
                                    idf[:],
                                )
                                stage = sp.tile([TQ, P], f32, tag="ystage")
                                nc.vector.tensor_copy(stage[:], ps[:])
                                nc.sync.dma_start(
                                    y1_d[
                                        b,
                                        t0 + tt * TQ:t0 + (tt + 1) * TQ,
                                        c * P:(c + 1) * P,
                                    ],
                                    stage[:],
                                )

            scan_ctx.close()
            dump_state(l)

    return nc


def _get_program():
    key = "full"
    if key not in _CACHE:
        _patch_walrus_passes()
        nc = build_nc()
        nc.finalize()
        _CACHE[key] = nc
    return _CACHE[key]


def kernel(**inputs):
    global LAST_RESULT
    _patch_walrus_passes()
    from concourse.bass_utils import run_bass_kernel_spmd

    ins = {k: np.asarray(v) for k, v in inputs.items()}
    x = ins["x"].astype(np.int32)

    nc = _get_program()
    rep = {}
    for name in ("emb", "Wg0", "bg0", "Wc0", "bc0", "Wg1", "bg1", "Wc1", "bc1"):
        rep[name] = np.ascontiguousarray(ins[name].astype(np.float32))
    in_maps = []
    for c in range(NCORES):
        sl = slice(c * BC, (c + 1) * BC)
        m = dict(rep)
        m["x"] = np.ascontiguousarray(x[sl])
        m["h0_0"] = np.ascontiguousarray(ins["h0_0"][sl].astype(np.float32))
        m["h0_1"] = np.ascontiguousarray(ins["h0_1"][sl].astype(np.float32))
        in_maps.append(m)

    res = run_bass_kernel_spmd(nc, in_maps, list(range(NCORES)), trace=TRACE)
    LAST_RESULT = res
    y1 = np.concatenate([res.results[c]["y1"] for c in range(NCORES)], axis=0)
    s0 = np.concatenate([res.results[c]["s0"] for c in range(NCORES)], axis=0)
    s1 = np.concatenate([res.results[c]["s1"] for c in range(NCORES)], axis=0)
    return y1, s0, s1


# revision 19
# speedup vs baseline: 1.1190x; 1.0864x over previous
"""Trainium2 Bass kernel for a 2-layer GRU encoder (TF GRUCell semantics).

Strategy: data-parallel over batch across 8 NeuronCores (4 rows/core, no
collectives — collective latency floors dwarf the per-step budget).

Per core:
- Embedding rows are gathered with indirect DMA, PE-transposed to a
  feature-major layout, and split into bf16 hi/lo pairs.
- Input projections (x_t @ W_x + b) are hoisted out of the scan and
  computed as large GEMMs into DRAM scratch (fp32).
- The sequential scan keeps the recurrent weights resident in SBUF as
  bf16 hi/lo pairs and computes each gate matmul as a 3-term split
  product (W_hi*h_hi + W_hi*h_lo + W_lo*h_hi, fp32 PSUM accumulation),
  which tracks the fp32 reference to ~1e-3 — plain bf16 diverges ~70%
  because this GRU amplifies per-step noise ~1000x over 512 steps, and
  native fp32 matmuls measured 427ns vs 34ns for bf16 pairs.
- The hidden state stays transposed (features on partitions) so the gate
  math uses all 128 DVE/ACT lanes.
- Dynamic (loop-variable) addressing appears only in whole-iteration slab
  DMAs: each engine has a tiny register budget and register-offset APs
  exhaust it fast.

Toolchain notes (pinned walrus build):
- at most ONE sync-wait per instruction -> split extras into NoOps.
- the lower_dve pass crashes on Tile SWDGE programs -> dropped (no
  custom DVE ops used).
- DVE instructions may read at most one PSUM operand.
"""

import numpy as np

B, S, V, E, U = 32, 512, 32000, 256, 1024
NCORES = 8
BC = B // NCORES          # batch rows per core
KH = U // 128             # 8  k-chunks of the hidden dim
KG = 2 * U // 128         # 16 m-chunks of the gate dim
KC = U // 128             # 8  m-chunks of the candidate dim
KE = E // 128             # 2  k-chunks of the embedding dim

_CACHE = {}
TRACE = False
LAST_RESULT = None


# ---------------------------------------------------------------------------
# toolchain workarounds
# ---------------------------------------------------------------------------

def _split_multiwait(nc, max_waits=1):
    """This walrus build accepts at most one sync-wait per instruction;
    split Tile's multi-wait instructions into single-wait NoOps."""
    import concourse.mybir as mybir

    n = 0
    for f in nc.m.functions:
        for bb in f.blocks:
            il = bb.instructions
            i = 0
            while i < len(il):
                inst = il[i]
                si = inst.sync_info
                if (
                    si is not None
                    and si.on_wait is not None
                    and len(si.on_wait) > max_waits
                ):
                    waits = list(si.on_wait)
                    keep = waits[-max_waits:]
                    extra = waits[:-max_waits]
                    nops = []
                    for w in extra:
                        n += 1
                        nop = mybir.InstNoOp(
                            name=f"I-wsplit{n}", engine=inst.engine, ins=[], outs=[]
                        )
                        nop.sync_info = mybir.SyncInfo(on_wait=[w], on_update=[])
                        nops.append(nop)
                    inst.sync_info = mybir.SyncInfo(
                        on_wait=keep, on_update=list(si.on_update or [])
                    )
                    for j, nop in enumerate(nops):
                        il.insert(i + j, nop)
                    i += len(nops)
                i += 1
    return n


_PASSES_PATCHED = False


def _patch_walrus_passes():
    """Drop the lower_dve walrus pass (crashes on Tile SWDGE programs; a
    no-op here since no custom DVE ops are used)."""
    global _PASSES_PATCHED
    if _PASSES_PATCHED:
        return
    _PASSES_PATCHED = True
    from concourse import bass_utils

    orig = bass_utils.bir_verify_and_optimise
    fn = orig.__wrapped__ if hasattr(orig, "__wrapped__") else orig

    def patched(tmpdir, inp="bir.json", outp="file.neff", arch=None, *, dve_root=None):
        orig_run = bass_utils.run_command

        def run_patched(cmd, **kw):
            cmd = [
                c.replace("lower_dve,", "") if isinstance(c, str) else c for c in cmd
            ]
            return orig_run(cmd, **kw)

        bass_utils.run_command = run_patched
        try:
            return fn(tmpdir, inp=inp, outp=outp, arch=arch, dve_root=dve_root)
        finally:
            bass_utils.run_command = orig_run

    bass_utils.bir_verify_and_optimise = patched


# ---------------------------------------------------------------------------
# program builder
# ---------------------------------------------------------------------------

def build_nc(S=S, V=V, unroll=16, nquarters=4, debug=False):
    import concourse.bass as bass
    import concourse.mybir as mybir
    import concourse.tile as tile
    from concourse import bacc
    from concourse.bass import ds
    from concourse.masks import make_identity

    dt = mybir.dt
    f32, bf16 = dt.float32, dt.bfloat16
    P = 128
    QS = S // nquarters
    assert QS % unroll == 0 and S % nquarters == 0
    HCH = min(P, S)        # hoist token-chunk (HCH steps x BC batch cols)
    SIG = mybir.ActivationFunctionType.Sigmoid
    TANH = mybir.ActivationFunctionType.Tanh

    nc = bacc.Bacc("TRN2", target_bir_lowering=False, debug=debug)

    x_d = nc.dram_tensor("x", [BC, S], dt.int32, kind="ExternalInput")
    emb_d = nc.dram_tensor("emb", [V, E], dt.float32, kind="ExternalInput")
    h0_d = [
        nc.dram_tensor("h0_0", [BC, U], dt.float32, kind="ExternalInput"),
        nc.dram_tensor("h0_1", [BC, U], dt.float32, kind="ExternalInput"),
    ]
    Wg_d = [
        nc.dram_tensor("Wg0", [E + U, 2 * U], dt.float32, kind="ExternalInput"),
        nc.dram_tensor("Wg1", [2 * U, 2 * U], dt.float32, kind="ExternalInput"),
    ]
    bg_d = [
        nc.dram_tensor("bg0", [2 * U], dt.float32, kind="ExternalInput"),
        nc.dram_tensor("bg1", [2 * U], dt.float32, kind="ExternalInput"),
    ]
    Wc_d = [
        nc.dram_tensor("Wc0", [E + U, U], dt.float32, kind="ExternalInput"),
        nc.dram_tensor("Wc1", [2 * U, U], dt.float32, kind="ExternalInput"),
    ]
    bc_d = [
        nc.dram_tensor("bc0", [U], dt.float32, kind="ExternalInput"),
        nc.dram_tensor("bc1", [U], dt.float32, kind="ExternalInput"),
    ]
    y1_d = nc.dram_tensor("y1", [BC, S, U], dt.float32, kind="ExternalOutput")
    s_d = [
        nc.dram_tensor("s0", [BC, U], dt.float32, kind="ExternalOutput"),
        nc.dram_tensor("s1", [BC, U], dt.float32, kind="ExternalOutput"),
    ]
    # DRAM scratch: hoisted projections + layer-0 outputs, feature-major
    gx_d = nc.dram_tensor("gx_scr", [P, KG, S, BC], dt.float32)
    cx_d = nc.dram_tensor("cx_scr", [P, KC, S, BC], dt.float32)
    y0_d = nc.dram_tensor("y0_scr", [P, KC, S, BC], dt.float32)

    from contextlib import ExitStack

    with tile.TileContext(nc) as tc, ExitStack() as ctx:
        const = ctx.enter_context(tc.tile_pool(name="const", bufs=1))
        wpool = ctx.enter_context(tc.tile_pool(name="w", bufs=1))
        sp = ctx.enter_context(tc.tile_pool(name="sp", bufs=2))
        qpool = ctx.enter_context(tc.tile_pool(name="q", bufs=1))
        sp3 = ctx.enter_context(tc.tile_pool(name="sp3", bufs=3))
        pp = ctx.enter_context(tc.tile_pool(name="pp", bufs=2, space="PSUM"))
        # pp carries only the small "ptr" transpose tiles; matmul psum tiles
        # live in per-phase pools so the 8 PSUM banks are never oversubscribed

        idf = const.tile([P, P], f32, tag="idf")
        make_identity(nc, idf[:])

        # persistent state
        xep = const.tile([P, KE, 2, S, BC], bf16, tag="xep")   # hi/lo pairs
        hT = const.tile([P, KH, 1, BC], f32, tag="hT")
        hpair = const.tile([P, KH, 2, BC], bf16, tag="hpair")

        # ---- biases, transposed to [128, nm] ----
        def load_bias_T(dram, nm, tag):
            stage = sp.tile([nm, P], f32, tag="bstage")
            nc.sync.dma_start(stage[:], dram[:])
            ps = pp.tile([P, nm], f32, tag="ptr")
            nc.tensor.transpose(ps[:], stage[:], idf[0:nm, 0:nm])
            out = const.tile([P, nm], f32, tag=tag)
            nc.vector.tensor_copy(out[:], ps[:])
            return out

        bgT = [load_bias_T(bg_d[l], KG, f"bgT{l}") for l in (0, 1)]
        bcT = [load_bias_T(bc_d[l], KC, f"bcT{l}") for l in (0, 1)]

        # ---- embedding gather + transpose + split into xep ----
        TT = min(P, S)
        for b in range(BC):
            for g in range(S // TT):
                t0 = g * TT
                idx = sp3.tile([TT, 1], dt.int32, tag="idx")
                nc.sync.dma_start(idx[:], x_d[b, t0:t0 + TT])
                xe = sp3.tile([TT, E], f32, tag="xe")
                nc.gpsimd.indirect_dma_start(
                    out=xe[:],
                    out_offset=None,
                    in_=emb_d[:],
                    in_offset=bass.IndirectOffsetOnAxis(ap=idx[:, :1], axis=0),
                )
                for e in range(KE):
                    ps = pp.tile([P, TT], f32, tag="ptr")
                    nc.tensor.transpose(
                        ps[:], xe[:, e * P:(e + 1) * P], idf[0:TT, 0:TT]
                    )
                    hi = xep[:, e, 0, t0:t0 + TT, b]
                    nc.vector.tensor_copy(hi, ps[:])
                    nc.vector.tensor_sub(xep[:, e, 1, t0:t0 + TT, b], ps[:], hi)

        # ---- split-weight loader: DRAM f32 rows -> bf16 hi/lo tiles ----
        def load_wpair(dram, row0, nk, nm, tag):
            whi = wpool.tile([P, nk, nm, P], bf16, tag=f"{tag}_hi")
            wlo = wpool.tile([P, nk, nm, P], bf16, tag=f"{tag}_lo")
            for k in range(nk):
                stage = qpool.tile([P, nm, P], f32, tag="wstage")
                r = row0 + k * P
                nc.sync.dma_start(stage[:], dram[r:r + P, :])
                nc.vector.tensor_copy(whi[:, k], stage[:])
                nc.vector.tensor_sub(wlo[:, k], stage[:], whi[:, k])
            return whi, wlo

        def h_init(l):
            stage = qpool.tile([BC, U], f32, tag="hstage")
            nc.sync.dma_start(stage[:], h0_d[l][:])
            for c in range(KH):
                ps = pp.tile([P, BC], f32, tag="ptr")
                nc.tensor.transpose(
                    ps[:], stage[:, c * P:(c + 1) * P], idf[0:BC, 0:BC]
                )
                nc.vector.tensor_copy(hT[:, c, 0, :], ps[:])
            nc.vector.tensor_copy(hpair[:, :, 0:1, :], hT[:])
            nc.vector.tensor_sub(hpair[:, :, 1:2, :], hT[:], hpair[:, :, 0:1, :])

        def dump_state(l):
            stage = qpool.tile([BC, KH, P], f32, tag="sstage")
            for c in range(KH):
                ps = pp.tile([BC, P], f32, tag="ptr")
                nc.tensor.transpose(ps[:], hT[:, c, 0, :], idf[:])
                nc.vector.tensor_copy(stage[:, c, :], ps[:])
            nc.sync.dma_start(s_d[l][:], stage[:])

        def mm3(psum, whi, wlo, rp, k, m, first, last):
            """psum[:, :] += Whi.T @ (hi|lo) + Wlo.T @ hi for one k-chunk.
            rp: [P, nk, 2, N...] bf16 pair source; psum cols packed [2, N]."""
            nc.tensor.matmul(
                psum[0], whi[:, k, m, :], rp[0],
                start=first, stop=False, skip_group_check=True,
            )
            nc.tensor.matmul(
                psum[1], wlo[:, k, m, :], rp[1],
                start=False, stop=last, skip_group_check=True,
            )

        for l in (0, 1):
            nin = E if l == 0 else U
            nkin = nin // P

            # ============ hoist x_t @ W_x + b for the whole sequence ====
            wxhi, wxlo = load_wpair(Wg_d[l], 0, nkin, KG, "wa")
            cxhi, cxlo = load_wpair(Wc_d[l], 0, nkin, KC, "wb")
            h_init(l)

            hoist_ctx = ExitStack()
            pph = hoist_ctx.enter_context(
                tc.tile_pool(name="pph", bufs=2, space="PSUM")
            )
            for hc in range(S // HCH):
                t0 = hc * HCH
                if l == 1:
                    y0q = qpool.tile([P, KH, HCH, BC], f32, tag="y0q")
                    nc.sync.dma_start(
                        y0q[:], y0_d[:, :, t0:t0 + HCH, :]
                    )
                    y0p = qpool.tile([P, KH, 2, HCH, BC], bf16, tag="y0p")
                    nc.vector.tensor_copy(y0p[:, :, 0, :, :], y0q[:])
                    nc.vector.tensor_sub(
                        y0p[:, :, 1, :, :], y0q[:], y0p[:, :, 0, :, :]
                    )
                for (wh, wl, nm, bias, dst) in (
                    (wxhi, wxlo, KG, bgT[l], gx_d),
                    (cxhi, cxlo, KC, bcT[l], cx_d),
                ):
                    for m in range(nm):
                        ph = pph.tile([P, HCH, BC], f32, tag="ph")
                        for k in range(nkin):
                            if l == 0:
                                rhi = xep[:, k, 0, t0:t0 + HCH, :]
                                rlo = xep[:, k, 1, t0:t0 + HCH, :]
                            else:
                                rhi = y0p[:, k, 0, :, :]
                                rlo = y0p[:, k, 1, :, :]
                            first = k == 0
                            last = k == nkin - 1
                            nc.tensor.matmul(
                                ph[:], wh[:, k, m, :], rhi,
                                start=first, stop=False, skip_group_check=True,
                            )
                            nc.tensor.matmul(
                                ph[:], wh[:, k, m, :], rlo,
                                start=False, stop=False, skip_group_check=True,
                            )
                            nc.tensor.matmul(
                                ph[:], wl[:, k, m, :], rhi,
                                start=False, stop=last, skip_group_check=True,
                            )
                        stage = sp.tile([P, HCH, BC], f32, tag="hstg")
                        nc.vector.scalar_tensor_tensor(
                            out=stage[:],
                            in0=ph[:],
                            scalar=1.0,
                            in1=bias[:, m:m + 1].to_broadcast([P, HCH, BC]),
                            op0=mybir.AluOpType.mult,
                            op1=mybir.AluOpType.add,
                        )
                        nc.sync.dma_start(dst[:, m, t0:t0 + HCH, :], stage[:])

            hoist_ctx.close()

            # ============ recurrent scan ================================
            ghi, glo = load_wpair(Wg_d[l], nin, KH, KG, "wa")
            chi, clo = load_wpair(Wc_d[l], nin, KH, KC, "wb")

            scan_ctx = ExitStack()
            pps = scan_ctx.enter_context(
                tc.tile_pool(name="pps", bufs=2, space="PSUM")
            )
            for q in range(nquarters):
                t0 = q * QS
                if l == 1:
                    y1q = qpool.tile([P, KC, QS, BC], f32, tag="y1q")
                with tc.For_i(
                    0, QS, unroll, hint_engines=(mybir.EngineType.PE,)
                ) as iv:
                    gx8 = sp.tile([P, KG, unroll, BC], f32, tag="gx8")
                    nc.sync.dma_start(
                        gx8[:], gx_d[:, :, ds(t0 + iv, unroll), :]
                    )
                    cx8 = sp.tile([P, KC, unroll, BC], f32, tag="cx8")
                    nc.sync.dma_start(
                        cx8[:], cx_d[:, :, ds(t0 + iv, unroll), :]
                    )
                    y8 = sp.tile([P, KC, unroll, BC], f32, tag="y8")
                    for ui in range(unroll):
                        # r and u halves in separate PSUM banks so the u-half
                        # matmuls don't serialize against the DVE reads of the
                        # r-half (same-bank PE-write/DVE-read is serialized)
                        pgr = pps.tile([P, KC, 2, BC], f32, tag="pgr")
                        for m in range(KC):          # r-half
                            for k in range(KH):
                                mm3(
                                    (pgr[:, m, :, :], pgr[:, m, 0:1, :]),
                                    ghi, glo,
                                    (hpair[:, k, :, :], hpair[:, k, 0:1, :]),
                                    k, m, k == 0, k == KH - 1,
                                )
                        pgu = pps.tile([P, KC, 2, BC], f32, tag="pgu")
                        for m in range(KC, KG):      # u-half
                            for k in range(KH):
                                mm3(
                                    (pgu[:, m - KC, :, :], pgu[:, m - KC, 0:1, :]),
                                    ghi, glo,
                                    (hpair[:, k, :, :], hpair[:, k, 0:1, :]),
                                    k, m, k == 0, k == KH - 1,
                                )
                        t1 = sp.tile([P, KC, 1, BC], f32, tag="t1")
                        nc.vector.tensor_add(
                            t1[:], pgr[:, :, 1:2, :],
                            gx8[:, 0:KC, ui:ui + 1, :],
                        )
                        gir = sp.tile([P, KC, 1, BC], f32, tag="gir")
                        nc.vector.tensor_add(gir[:], pgr[:, :, 0:1, :], t1[:])
                        rg = sp.tile([P, KC, 1, BC], f32, tag="rg")
                        nc.scalar.activation(rg[:], gir[:], SIG)
                        rh = sp.tile([P, KH, 1, BC], f32, tag="rh")
                        nc.vector.tensor_mul(rh[:], rg[:], hT[:])
                        rp = sp.tile([P, KH, 2, BC], bf16, tag="rp")
                        nc.vector.tensor_copy(rp[:, :, 0:1, :], rh[:])
                        rp_lo = nc.vector.tensor_sub(
                            rp[:, :, 1:2, :], rh[:], rp[:, :, 0:1, :]
                        )
                        # u-gate reduction ordered AFTER the r chain on DVE:
                        # the cost model has zero LDWEIGHTS cost, so without
                        # the explicit edge the scheduler queues these first
                        # and the in-order DVE stalls on the u-half PSUM.
                        t2 = sp.tile([P, KC, 1, BC], f32, tag="t2")
                        t2i = nc.vector.tensor_add(
                            t2[:], pgu[:, :, 1:2, :],
                            gx8[:, KC:KG, ui:ui + 1, :],
                        )
                        bass._add_dep_helper(
                            t2i.ins, rp_lo.ins, False, "dve order: r chain first"
                        )
                        giu = sp.tile([P, KC, 1, BC], f32, tag="giu")
                        nc.vector.tensor_add(giu[:], pgu[:, :, 0:1, :], t2[:])
                        ug = sp.tile([P, KC, 1, BC], f32, tag="ug")
                        nc.scalar.activation(ug[:], giu[:], SIG)
                        um = sp.tile([P, KC, 1, BC], f32, tag="um")
                        nc.scalar.activation(um[:], giu[:], SIG, scale=-1.0)
                        uh = sp.tile([P, KC, 1, BC], f32, tag="uh")
                        nc.vector.tensor_mul(uh[:], ug[:], hT[:])   # during c-mms
                        psc = pps.tile([P, KC, 2, BC], f32, tag="pc")
                        for m in range(KC):          # candidate
                            for k in range(KH):
                                mm3(
                                    (psc[:, m, :, :], psc[:, m, 0:1, :]),
                                    chi, clo,
                                    (rp[:, k, :, :], rp[:, k, 0:1, :]),
                                    k, m, k == 0, k == KH - 1,
                                )
                        t3 = sp.tile([P, KC, 1, BC], f32, tag="t3")
                        nc.vector.tensor_add(
                            t3[:], psc[:, :, 1:2, :], cx8[:, :, ui:ui + 1, :]
                        )
                        cpre = sp.tile([P, KC, 1, BC], f32, tag="cpre")
                        nc.vector.tensor_add(cpre[:], psc[:, :, 0:1, :], t3[:])
                        cg = sp.tile([P, KC, 1, BC], f32, tag="cg")
                        nc.scalar.activation(cg[:], cpre[:], TANH)
                        # h' = u*h + (1-u)*c  (u*h precomputed above)
                        t5 = sp.tile([P, KC, 1, BC], f32, tag="t5")
                        nc.vector.tensor_mul(t5[:], um[:], cg[:])
                        nc.vector.tensor_add(hT[:], uh[:], t5[:])
                        hp_hi = nc.vector.tensor_copy(hpair[:, :, 0:1, :], hT[:])
                        hp_lo = nc.vector.tensor_sub(
                            hpair[:, :, 1:2, :], hT[:], hpair[:, :, 0:1, :]
                        )
                        y8i = nc.vector.tensor_copy(y8[:, :, ui:ui + 1, :], hT[:])
                        bass._add_dep_helper(
                            y8i.ins, hp_lo.ins, False, "y8 after hpair"
                        )
                    if l == 0:
                        nc.sync.dma_start(
                            y0_d[:, :, ds(t0 + iv, unroll), :], y8[:]
                        )
                    else:
                        nc.sync.dma_start(
                            y1q[:, :, ds(iv, unroll), :], y8[:]
                        )

                # ---- dump y1 quarter: transpose back to [b, t, u] ----
                if l == 1:
                    TQ = min(P, QS)
                    for c in range(KC):
                        for b in range(BC):
                            for tt in range(QS // TQ):
                                ps = pp.tile([TQ, P], f32, tag="ptr")
                                nc.tensor.transpose(
                                    ps[:],
                                    y1q[:, c, tt * TQ:(tt + 1) * TQ, b],
                                    idf[:],
                                )
                                stage = sp.tile([TQ, P], f32, tag="ystage")
                                nc.vector.tensor_copy(stage[:], ps[:])
                                nc.sync.dma_start(
                                    y1_d[
                                        b,
                                        t0 + tt * TQ:t0 + (tt + 1) * TQ,
                                        c * P:(c + 1) * P,
                                    ],
                                    stage[:],
                                )

            scan_ctx.close()
            dump_state(l)

    return nc


def _get_program():
    key = "full"
    if key not in _CACHE:
        _patch_walrus_passes()
        nc = build_nc()
        nc.finalize()
        _CACHE[key] = nc
    return _CACHE[key]


def kernel(**inputs):
    global LAST_RESULT
    _patch_walrus_passes()
    from concourse.bass_utils import run_bass_kernel_spmd

    ins = {k: np.asarray(v) for k, v in inputs.items()}
    x = ins["x"].astype(np.int32)

    nc = _get_program()
    rep = {}
    for name in ("emb", "Wg0", "bg0", "Wc0", "bc0", "Wg1", "bg1", "Wc1", "bc1"):
        rep[name] = np.ascontiguousarray(ins[name].astype(np.float32))
    in_maps = []
    for c in range(NCORES):
        sl = slice(c * BC, (c + 1) * BC)
        m = dict(rep)
        m["x"] = np.ascontiguousarray(x[sl])
        m["h0_0"] = np.ascontiguousarray(ins["h0_0"][sl].astype(np.float32))
        m["h0_1"] = np.ascontiguousarray(ins["h0_1"][sl].astype(np.float32))
        in_maps.append(m)

    res = run_bass_kernel_spmd(nc, in_maps, list(range(NCORES)), trace=TRACE)
    LAST_RESULT = res
    y1 = np.concatenate([res.results[c]["y1"] for c in range(NCORES)], axis=0)
    s0 = np.concatenate([res.results[c]["s0"] for c in range(NCORES)], axis=0)
    s1 = np.concatenate([res.results[c]["s1"] for c in range(NCORES)], axis=0)
    return y1, s0, s1
